# revision 2
# baseline (speedup 1.0000x reference)
"""Trainium2 Bass kernel for nn_LongThinNet (16-layer thin MLP, batch 2^20).

Strategy (pure data parallelism, batch sharded 8 ways):
  Per core 131072 rows. Feature-major compute: activations live as
  [feature-partitions, batch-free] tiles so the 10-wide layers chain through
  the PE array with block-diagonal weights (12 batch-interleaved j-slices
  packed across 128 partitions in 4 x 32-aligned bands of 3 slices each).

  - Input: flat [128, 5120] tiles (partition p = 32 consecutive rows of a
    4096-row group; host pre-interleaves so each partition line is 20KB
    contiguous DRAM). PE transpose (fp32, is_transpose) of 120-wide j-triple
    slices produces feature-major xT tiles.
  - Layers: one full-width matmul per layer per tile (N=512 over 4 groups),
    activation = Prelu(alpha=0.5) == 0.5x + 0.5relu(x) with fused
    per-partition bias, PSUM->SBUF on the scalar engine (some layers on DVE
    via tensor_scalar + scalar_tensor_tensor to balance engines).
  - Final layer: h as stationary operand, block-diag W_out.T as moving ->
    batch-packed [128, 320] PSUM (32 rows/partition, 1280B DMA lines);
    b_out folded in via a constant-1 pad row produced by the last act's bias.
"""

import sys

sys.path.insert(0, "/opt/trn_rl_repo")

from contextlib import ExitStack

import numpy as np

import concourse.bass as bass
import concourse.mybir as mybir
import concourse.tile as tile
from concourse.bass_utils import run_bass_kernel_spmd

F32 = mybir.dt.float32
AF = mybir.ActivationFunctionType
ALU = mybir.AluOpType

NCORES = 8
BC = 131072          # rows per core
IN, HID = 40, 10
NMID = 14            # middle 10->10 layers
SG = 8               # supergroups per core, 16384 rows each
DVE_ACT_LAYERS = frozenset({4, 9})  # act layers (0..14) done on DVE

# (beta, i, j_local) bands: A/B tiles have 12 j-slices, C has 8
BANDS_AB = [(b, i, 3 * b + i) for b in range(4) for i in range(3)]
BANDS_C = [(b, i, 3 * b + i) for b in range(2) for i in range(3)] + [
    (2, i, 6 + i) for i in range(2)
]


def _skip(name):
    return name in ("InstEventSemaphore", "InstAllEngineBarrier")


def _split_multi_waits(nc):
    """walrus codegen allows <=1 semaphore wait per instruction; hoist extras
    onto standalone InstEventSemaphore instructions inserted just before."""
    n_new = 0
    for f in nc.m.functions:
        for bb in f.blocks:
            out, changed = [], False
            for inst in bb.instructions:
                si = inst.sync_info
                if si is not None and len(si.on_wait) > 1 and not _skip(type(inst).__name__):
                    waits = list(si.on_wait)
                    for w in waits[:-1]:
                        n_new += 1
                        out.append(
                            mybir.InstEventSemaphore(
                                name=f"EVW-{n_new}-{inst.name}",
                                engine=inst.engine,
                                sync_info=mybir.SyncInfo(on_wait=[w], on_update=[]),
                            )
                        )
                    inst.sync_info = mybir.SyncInfo(
                        on_wait=[waits[-1]], on_update=list(si.on_update)
                    )
                    changed = True
                out.append(inst)
            if changed:
                try:
                    bb.instructions = out
                except Exception:
                    lst = bb.instructions
                    lst.clear()
                    lst.extend(out)
    return n_new


def _pack_weights(W_in, b_in, W_mid, b_mid, W_out, b_out):
    wl0 = np.zeros((120, 32), np.float32)
    for g in range(3):
        wl0[40 * g:40 * g + 40, 10 * g:10 * g + 10] = W_in.T
    wl0c = np.zeros((80, 20), np.float32)
    for g in range(2):
        wl0c[40 * g:40 * g + 40, 10 * g:10 * g + 10] = W_in.T

    wmid = np.zeros((128, NMID * 128), np.float32)
    wmidc = np.zeros((84, NMID * 84), np.float32)
    for l in range(NMID):
        for b, i, _ in BANDS_AB:
            q = 32 * b + 10 * i
            wmid[q:q + 10, 128 * l + q:128 * l + q + 10] = W_mid[l].T
        for b, i, _ in BANDS_C:
            q = 32 * b + 10 * i
            wmidc[q:q + 10, 84 * l + q:84 * l + q + 10] = W_mid[l].T

    wl15 = np.zeros((128, 120), np.float32)
    for b, i, jl in BANDS_AB:
        q = 32 * b + 10 * i
        wl15[q:q + 10, 10 * jl:10 * jl + 10] = W_out.T
        wl15[30, 10 * jl:10 * jl + 10] = b_out
    wl15c = np.zeros((84, 80), np.float32)
    for b, i, jl in BANDS_C:
        q = 32 * b + 10 * i
        wl15c[q:q + 10, 10 * jl:10 * jl + 10] = W_out.T
        wl15c[30, 10 * jl:10 * jl + 10] = b_out

    wbias = np.zeros((128, 16), np.float32)
    for b, i, _ in BANDS_AB:
        q = 32 * b + 10 * i
        wbias[q:q + 10, 0] = b_in
        for l in range(NMID):
            wbias[q:q + 10, 1 + l] = b_mid[l]
    wbias[30, NMID] = 1.0  # constant-1 row for folding b_out into L15

    ident = np.eye(128, dtype=np.float32)
    return {"wl0": wl0, "wl0c": wl0c, "wmid": wmid, "wmidc": wmidc,
            "wl15": wl15, "wl15c": wl15c, "wbias": wbias, "ident": ident}


def _act_tile(nc, su_pool, dst, psum, bias_ap, on_dve, width):
    """dst = Prelu(psum + bias, alpha=0.5) over [128, width]."""
    if not on_dve:
        nc.scalar.activation(dst, psum, AF.Prelu, bias=bias_ap, scale=1.0, alpha=0.5)
    else:
        u = su_pool.tile([128, width], F32, tag=f"u{width}")
        nc.vector.tensor_scalar(u[:], psum, bias_ap, 0.5, ALU.add, ALU.mult)
        nc.vector.scalar_tensor_tensor(dst, psum, bias_ap, u[:], ALU.add, ALU.max)


def _build_nc():
    nc = bass.Bass("TRN2", target_bir_lowering=False, debug=False)

    x_d = nc.dram_tensor("x", [SG, 128, 5120], F32, kind="ExternalInput").ap()
    wl0_d = nc.dram_tensor("wl0", [120, 32], F32, kind="ExternalInput").ap()
    wl0c_d = nc.dram_tensor("wl0c", [80, 20], F32, kind="ExternalInput").ap()
    wmid_d = nc.dram_tensor("wmid", [128, NMID * 128], F32, kind="ExternalInput").ap()
    wmidc_d = nc.dram_tensor("wmidc", [84, NMID * 84], F32, kind="ExternalInput").ap()
    wl15_d = nc.dram_tensor("wl15", [128, 120], F32, kind="ExternalInput").ap()
    wl15c_d = nc.dram_tensor("wl15c", [84, 80], F32, kind="ExternalInput").ap()
    wbias_d = nc.dram_tensor("wbias", [128, 16], F32, kind="ExternalInput").ap()
    ident_d = nc.dram_tensor("ident", [128, 128], F32, kind="ExternalInput").ap()
    out_d = nc.dram_tensor("out", [SG, 128, 1280], F32, kind="ExternalOutput").ap()

    with tile.TileContext(nc) as tc, ExitStack() as ctx:
        sc = ctx.enter_context(tc.tile_pool(name="sc", bufs=1))
        sx = ctx.enter_context(tc.tile_pool(name="sx", bufs=2))
        sxT = ctx.enter_context(tc.tile_pool(name="sxT", bufs=4))
        shab = ctx.enter_context(tc.tile_pool(name="shab", bufs=3))
        shc = ctx.enter_context(tc.tile_pool(name="shc", bufs=3))
        su = ctx.enter_context(tc.tile_pool(name="su", bufs=2))
        sout = ctx.enter_context(tc.tile_pool(name="sout", bufs=2))
        pxT = ctx.enter_context(tc.tile_pool(name="pxT", bufs=2, space="PSUM"))
        phab = ctx.enter_context(tc.tile_pool(name="phab", bufs=2, space="PSUM"))
        phc = ctx.enter_context(tc.tile_pool(name="phc", bufs=1, space="PSUM"))
        pout = ctx.enter_context(tc.tile_pool(name="pout", bufs=1, space="PSUM"))

        consts = {}
        for name, dram, shape in [
            ("wl0", wl0_d, [120, 32]), ("wl0c", wl0c_d, [80, 20]),
            ("wmid", wmid_d, [128, NMID * 128]), ("wmidc", wmidc_d, [84, NMID * 84]),
            ("wl15", wl15_d, [128, 120]), ("wl15c", wl15c_d, [84, 80]),
            ("wbias", wbias_d, [128, 16]), ("ident", ident_d, [128, 128]),
        ]:
            t = sc.tile(shape, F32, tag=name)
            nc.sync.dma_start(t[:], dram)
            consts[name] = t

        def bias_ap(l):
            return consts["wbias"][:, l:l + 1]

        for sg in range(SG):
            x_sg = sx.tile([128, 5120], F32, tag="x")
            nc.sync.dma_start(x_sg[:], x_d[sg])

            # transposes + PSUM->SBUF copies: xT tiles per j-triple
            xT = []
            for t in range(11):
                w = 120 if t < 10 else 80
                p_t = pxT.tile([w, 512], F32, tag="xT")
                for g in range(4):
                    nc.tensor.transpose(
                        p_t[:, 128 * g:128 * g + 128],
                        x_sg[:, 1280 * g + 120 * t:1280 * g + 120 * t + w],
                        consts["ident"][:],
                    )
                s_t = sxT.tile([w, 512], F32, tag="xT")
                nc.vector.tensor_copy(s_t[:], p_t[:])
                xT.append(s_t)

            # L0: 40 -> 10, block-diag x3 into banded tiles
            p_ab = phab.tile([128, 1024], F32, tag="hab")
            p_c = phc.tile([128, 512], F32, tag="hc")
            for t in range(8):
                half, band = (0 if t < 4 else 512), 32 * (t % 4)
                nc.tensor.matmul(
                    p_ab[band:band + 32, half:half + 512],
                    consts["wl0"][:], xT[t][:], start=True, stop=True,
                    tile_position=(0, band),
                )
            for t in (8, 9):
                band = 32 * (t - 8)
                nc.tensor.matmul(
                    p_c[band:band + 32, :], consts["wl0"][:], xT[t][:],
                    start=True, stop=True, tile_position=(0, band),
                )
            nc.tensor.matmul(
                p_c[64:84, :], consts["wl0c"][:], xT[10][:], start=True, stop=True,
                tile_position=(0, 64),
            )
            s_ab = shab.tile([128, 1024], F32, tag="hab")
            s_c = shc.tile([128, 512], F32, tag="hc")
            dve = 0 in DVE_ACT_LAYERS
            _act_tile(nc, su, s_ab[:], p_ab[:], bias_ap(0), dve, 1024)
            _act_tile(nc, su, s_c[:], p_c[:], bias_ap(0), dve, 512)

            # 14 middle layers
            for l in range(NMID):
                p_ab = phab.tile([128, 1024], F32, tag="hab")
                p_c = phc.tile([128, 512], F32, tag="hc")
                wm = consts["wmid"][:, 128 * l:128 * l + 128]
                wmc = consts["wmidc"][0:84, 84 * l:84 * l + 84]
                nc.tensor.matmul(p_ab[:, 0:512], wm, s_ab[:, 0:512],
                                 start=True, stop=True)
                nc.tensor.matmul(p_ab[:, 512:1024], wm, s_ab[:, 512:1024],
                                 start=True, stop=True)
                nc.tensor.matmul(p_c[0:84, :], wmc, s_c[0:84, :],
                                 start=True, stop=True)
                s_ab = shab.tile([128, 1024], F32, tag="hab")
                s_c = shc.tile([128, 512], F32, tag="hc")
                dve = (l + 1) in DVE_ACT_LAYERS
                _act_tile(nc, su, s_ab[:], p_ab[:], bias_ap(l + 1), dve, 1024)
                _act_tile(nc, su, s_c[:], p_c[:], bias_ap(l + 1), dve, 512)

            # L15: h stationary -> batch-packed output [128, 320] per group
            s_o = sout.tile([128, 1280], F32, tag="out")
            for g in range(4):
                p_o = pout.tile([128, 320], F32, tag="pout")
                nc.tensor.matmul(p_o[:, 0:120], s_ab[:, 128 * g:128 * g + 128],
                                 consts["wl15"][:], start=True, stop=True)
                nc.tensor.matmul(p_o[:, 120:240],
                                 s_ab[:, 512 + 128 * g:512 + 128 * g + 128],
                                 consts["wl15"][:], start=True, stop=True)
                nc.tensor.matmul(p_o[:, 240:320],
                                 s_c[0:84, 128 * g:128 * g + 128],
                                 consts["wl15c"][:], start=True, stop=True)
                nc.vector.tensor_copy(s_o[:, 320 * g:320 * g + 320], p_o[:])
            nc.sync.dma_start(out_d[sg], s_o[:])

    _split_multi_waits(nc)
    return nc


_NC_CACHE = {}


def kernel(x, W_in, b_in, W_mid, b_mid, W_out, b_out):
    x = np.asarray(x, np.float32)
    W_in = np.asarray(W_in, np.float32)
    b_in = np.asarray(b_in, np.float32)
    W_mid = np.asarray(W_mid, np.float32)
    b_mid = np.asarray(b_mid, np.float32)
    W_out = np.asarray(W_out, np.float32)
    b_out = np.asarray(b_out, np.float32)

    if "nc" not in _NC_CACHE:
        _NC_CACHE["nc"] = _build_nc()
    nc = _NC_CACHE["nc"]

    consts = _pack_weights(W_in, b_in, W_mid, b_mid, W_out, b_out)

    in_maps = []
    for c in range(NCORES):
        xc = x[c * BC:(c + 1) * BC]
        # rows -> [sg][g][p][j][f] -> partition-major [sg][p][(g,j,f)]
        xc = xc.reshape(SG, 4, 128, 32, IN).transpose(0, 2, 1, 3, 4)
        xc = np.ascontiguousarray(xc).reshape(SG, 128, 5120)
        in_maps.append({"x": xc, **consts})

    res = run_bass_kernel_spmd(nc, in_maps, list(range(NCORES)))

    outs = []
    for c in range(NCORES):
        oc = res.results[c]["out"]  # [SG, 128, 1280] = [sg][p][(g,j,o)]
        oc = oc.reshape(SG, 128, 4, 32, HID).transpose(0, 2, 1, 3, 4)
        outs.append(oc.reshape(BC, HID))
    return np.ascontiguousarray(np.concatenate(outs, axis=0))
